# revision 52
# baseline (speedup 1.0000x reference)
"""CAM (channel-attention) + SE module kernel for TRN2, batch-parallel over 8 cores.

Per sample (C=256, N=9216):
  v = x.reshape(C, N)
  E = v @ v.T         energy: fp16 matmuls, fp32 PSUM accum; E is symmetric,
                      so only E00|E01 and E11 are computed and E10 = E01^T is
                      mirrored by one PE transpose in the epilogue
  a = exp(rowmin(E) - E)             (softmax numerator, fp16, from ACT exp)
  pooled = mean(x) over N            (free reduction riding the x-load cast)
  gate = sigmoid(w2 @ relu(w1 @ pooled + b1) + b2)   (computed early: only
                                      needs pooled; relu/sigmoid run on DVE
                                      to keep ACT tables off the preamble)
  as = a * (gamma*gate/rowsum(a))    (per-row scale folded into att, fp16)
  out = as @ v + x                   512-col slabs; 3 of 4 slabs drain from
                                     PSUM on the DVE with the +x residual
                                     fused into a scalar_tensor_tensor, the
                                     4th adds x on the PE (identity matmul)
                                     and drains with a plain ACT copy
  out is stored fp16 (halves store-side HBM traffic); the host upcasts.

All PE traffic is fp16 (1cyc/col streams, FWL-eligible weight loads). x lives
on-chip only as fp16 (rounded once during load on ACT, whose accum_out gives
pooled for free). Emission is a two-sample software pipeline: phase 1 runs
with a one-group lag (transposes of group i, matmuls of group i-1) so the PE
never waits on the DVE's PSUM->SBUF copy, and sample-0 phase-2 slab pairs are
woven between sample-1 phase-1 groups so every engine's in-order queue keeps
ready work at its head while load DMAs stream in.
"""
import numpy as np
import concourse.bass as bass
import concourse.bacc as bacc
import concourse.tile as tile
import concourse.mybir as mybir
from concourse.bass_utils import run_bass_kernel_spmd

F32 = mybir.dt.float32
F16 = mybir.dt.float16
F8 = mybir.dt.float8e4

B, C, H, W = 16, 256, 96, 96
N = H * W                 # 9216
NCORES = 8
BL = B // NCORES          # samples per core
NCH = N // 128            # 72 n-chunks for the energy phase
GRP = 4                   # chunks per phase-1 group (one PSUM bank of fp16)
NGRP = NCH // GRP         # 18
NT = 512                  # phase-2 matmul width (one PSUM bank of fp32)
PAIR = 1024               # phase-2 psum tile / out-DMA chunk (2 banks)
# load segments: small first chunks so phase-1 transposes start early
SEGS = [512, 1024, 1536, 1536, 1536, 1536, 1536]
NSEG = len(SEGS)
R = C // 8                # 32 (SE hidden dim)


def build_nc():
    nc = bacc.Bacc("TRN2", target_bir_lowering=False, debug=False, num_devices=NCORES)

    x_d = nc.dram_tensor("x", [BL, C, N], F32, kind="ExternalInput")
    gamma_d = nc.dram_tensor("gamma", [1], F32, kind="ExternalInput")
    w1_d = nc.dram_tensor("w1", [R, C], F32, kind="ExternalInput")   # pre-scaled by 1/N
    b1_d = nc.dram_tensor("b1", [R], F32, kind="ExternalInput")
    w2_d = nc.dram_tensor("w2", [C, R], F32, kind="ExternalInput")
    b2_d = nc.dram_tensor("b2", [C], F32, kind="ExternalInput")
    ident_d = nc.dram_tensor("ident", [128, 128], F32, kind="ExternalInput")
    # fp16 output: halves the store-side HBM traffic; the host upcasts.
    # fp16 rounding of out adds ~3e-4 rel err (budget is 2e-2).
    out_d = nc.dram_tensor("out", [BL, C, N], F16, kind="ExternalOutput")

    with tile.TileContext(nc) as tc:
        with (
            tc.tile_pool(name="px", bufs=2 * BL) as px,
            tc.tile_pool(name="pstage", bufs=4) as pstage,
            tc.tile_pool(name="pxT", bufs=3) as pxT,
            tc.tile_pool(name="patt", bufs=2) as patt,
            tc.tile_pool(name="pout", bufs=18) as pout,
            tc.tile_pool(name="psmall", bufs=2) as psmall,
            tc.tile_pool(name="psingle", bufs=1) as psingle,
            tc.tile_pool(name="ppsE", bufs=2, space="PSUM") as ppsE,
            tc.tile_pool(name="ppsX", bufs=2, space="PSUM") as ppsX,
            tc.tile_pool(name="ppsO", bufs=4, space="PSUM") as ppsO,
        ):
            EARLY_SEGS = 2

            # ---------------- parameter prep (once) ----------------
            ident = psingle.tile([128, 128], F32, name="ident")
            nc.gpsimd.dma_start(out=ident[:], in_=ident_d[:])
            ident16 = psingle.tile([128, 128], F16, name="ident16")
            nc.vector.tensor_copy(out=ident16[:], in_=ident[:])
            gamma_sb = psingle.tile([128, 1], F32, name="gamma_sb")
            nc.gpsimd.dma_start(
                out=gamma_sb[:],
                in_=bass.AP(tensor=gamma_d.ap().tensor, offset=0, ap=[[0, 128], [1, 1]]),
            )
            b1_sb = psingle.tile([R, 1], F32, name="b1_sb")
            nc.gpsimd.dma_start(
                out=b1_sb[:],
                in_=bass.AP(tensor=b1_d.ap().tensor, offset=0, ap=[[1, R], [1, 1]]),
            )
            b2_sb = psingle.tile([128, 2], F32, name="b2_sb")
            nc.gpsimd.dma_start(out=b2_sb[:], in_=b2_d[:].rearrange("(h c) -> c h", c=128))

            # w1T[c, h, r] = w1[r, h*128+c]
            w1_nat = psingle.tile([R, 2, 128], F32, name="w1_nat")
            nc.gpsimd.dma_start(out=w1_nat[:], in_=w1_d[:].rearrange("r (h c) -> r h c", c=128))
            w1T_ps = ppsX.tile([128, 2, R], F32, tag="psx", name="w1T_ps")
            for h in range(2):
                nc.tensor.transpose(w1T_ps[:, h, :], w1_nat[:, h, :], ident[0:R, 0:R])
            w1T = psingle.tile([128, 2, R], F32, name="w1T")
            nc.vector.tensor_copy(out=w1T[:], in_=w1T_ps[:])

            # w2T[r, h*128+c] = w2[h*128+c, r]
            w2_nat = psingle.tile([128, 2, R], F32, name="w2_nat")
            nc.gpsimd.dma_start(out=w2_nat[:], in_=w2_d[:].rearrange("(h c) r -> c h r", c=128))
            w2T = psingle.tile([R, 2, 128], F32, name="w2T")
            for h in range(2):
                w2T_ps = ppsX.tile([R, 128], F32, tag="psx", name=f"w2T_ps_{h}")
                nc.tensor.transpose(w2T_ps[:], w2_nat[:, h, :], ident[:])
                nc.vector.tensor_copy(out=w2T[:, h, :], in_=w2T_ps[:])

            # ---------------- per sample (software-pipelined) ----------------
            x16 = {}
            pp = {}
            psE = {}
            attT = {}
            attT8 = {}
            rs = {}
            gg = {}

            SEG_OFF = [sum(SEGS[:i]) for i in range(NSEG)]
            # first phase-1 group (4 chunks) that needs segment g
            SEG_FIRST_GROUP = {g: SEG_OFF[g] * NCH // N // GRP for g in range(NSEG)}

            def emit_load_start(b):
                pp[b] = psmall.tile([128, 2, NSEG], F32, tag="pp", name=f"pp_{b}")
                x16[b] = [
                    px.tile([128, N], F16, tag="x16", name=f"x_{b}_{h}")
                    for h in range(2)
                ]

            def emit_load_seg(b, g):
                seg = SEGS[g]
                sl = slice(SEG_OFF[g], SEG_OFF[g] + seg)
                for h in range(2):
                    st = pstage.tile([128, max(SEGS)], F32, tag="stage",
                                     name=f"st_{b}_{h}_{g}")
                    nc.sync.dma_start(
                        out=st[:, 0:seg], in_=x_d[b, 128 * h:128 * (h + 1), sl],
                    )
                    nc.scalar.activation(
                        out=x16[b][h][:, sl], in_=st[:, 0:seg],
                        func=mybir.ActivationFunctionType.Copy,
                        accum_out=pp[b][:, h, g:g + 1],
                    )

            # first x segments in flight before/while params load: the DMA
            # queues and ACT table load overlap the parameter prep
            emit_load_start(0)
            for g in range(EARLY_SEGS):
                emit_load_seg(0, g)

            def alloc_psE(b):
                psE[b] = ppsE.tile([128, 512], F32, tag="psE", name=f"psE_{b}")

            xTs = {}

            def emit_phase1_T(b, gi):
                # transposes for chunks k = GRP*gi .. +3 into one fp16 PSUM
                # bank, then a DVE copy to SBUF
                xT_ps = ppsX.tile([128, GRP, 256], F16, tag="psx", name=f"xTps_{b}_{gi}")
                for q in range(GRP):
                    k = GRP * gi + q
                    for h in range(2):
                        nc.tensor.transpose(
                            xT_ps[:, q, 128 * h:128 * (h + 1)],
                            x16[b][h][:, 128 * k:128 * (k + 1)],
                            ident16[:],
                        )
                xT = pxT.tile([128, GRP, 256], F16, tag="xT", name=f"xT_{b}_{gi}")
                nc.vector.tensor_copy(out=xT[:], in_=xT_ps[:])
                xTs[(b, gi)] = xT

            def emit_phase1_M(b, gi):
                # energy matmuls for group gi (E symmetric: h=1 computes only
                # the E11 block, E10 is mirrored from E01 in the epilogue)
                xT = xTs.pop((b, gi))
                last = (gi == NGRP - 1)
                for q in range(GRP):
                    # one accumulation group for the whole bank (start clears
                    # the bank-wide has_written flags, so regions must share
                    # a single start/stop); h=1 computes only the E11 block,
                    # E10 is mirrored from E01 in the epilogue
                    nc.tensor.matmul(
                        psE[b][:, 0:256],
                        xT[:, q, 0:128],
                        xT[:, q, :],
                        start=(gi == 0 and q == 0),
                        stop=False,
                        skip_group_check=True,
                    )
                    nc.tensor.matmul(
                        psE[b][:, 384:512],
                        xT[:, q, 128:256],
                        xT[:, q, 128:256],
                        start=False,
                        stop=(last and q == GRP - 1),
                        skip_group_check=True,
                    )

            def emit_se(b):
                # SE gate from pooled sums (w1 pre-scaled by 1/N on host);
                # only needs the load to be done, so runs well before softmax
                pooled = psmall.tile([128, 2], F32, tag="pooled", name=f"pooled_{b}")
                for h in range(2):
                    nc.vector.reduce_sum(
                        out=pooled[:, h:h + 1], in_=pp[b][:, h, :], axis=mybir.AxisListType.X,
                    )
                hid_ps = ppsX.tile([R, 1], F32, tag="psx", name=f"hid_ps_{b}")
                for h in range(2):
                    nc.tensor.matmul(
                        hid_ps[:], w1T[:, h, :], pooled[:, h:h + 1],
                        start=(h == 0), stop=(h == 1),
                    )
                # relu(hid + b1) on DVE (keeps the Relu table off the ACT preamble)
                hid = psmall.tile([R, 1], F32, tag="hid", name=f"hid_{b}")
                nc.vector.tensor_scalar(
                    out=hid[:], in0=hid_ps[:], scalar1=b1_sb[:], scalar2=0.0,
                    op0=mybir.AluOpType.add, op1=mybir.AluOpType.max,
                )
                gg[b] = psmall.tile([128, 2], F32, tag="gg", name=f"gg_{b}")
                for h in range(2):
                    gate_ps = ppsX.tile([128, 1], F32, tag="psx", name=f"gate_ps_{b}_{h}")
                    nc.tensor.matmul(gate_ps[:], w2T[:, h, :], hid[:])
                    # sigmoid(z + b2) = 1/(1 + exp(-z - b2)): Exp table + DVE;
                    # b2_sb holds -b2 (negated host-side)
                    ez = psmall.tile([128, 1], F32, tag="ez", name=f"ez_{b}_{h}")
                    nc.scalar.activation(
                        out=ez[:], in_=gate_ps[:],
                        func=mybir.ActivationFunctionType.Exp,
                        bias=b2_sb[:, h:h + 1], scale=-1.0,
                    )
                    nc.vector.tensor_scalar_add(out=ez[:], in0=ez[:], scalar1=1.0)
                    nc.vector.reciprocal(out=gg[b][:, h:h + 1], in_=ez[:])
                nc.vector.tensor_scalar_mul(out=gg[b][:], in0=gg[b][:], scalar1=gamma_sb[:])

            def emit_softmax(b):
                # mirror E10 = E01^T (E is symmetric; the h=1 energy matmuls
                # computed only the E11 block)
                e01 = psmall.tile([128, 128], F32, tag="e01", name=f"e01_{b}")
                nc.vector.tensor_copy(out=e01[:], in_=psE[b][:, 128:256])
                nc.tensor.transpose(psE[b][:, 256:384], e01[:], ident[:])

                # rows: unnormalized exp, then fold gamma*gate/rowsum into att
                att = []
                for h in range(2):
                    pE = psE[b][:, 256 * h:256 * (h + 1)]
                    mn = psmall.tile([128, 1], F32, tag="mn", name=f"mn_{b}_{h}")
                    nc.vector.tensor_reduce(
                        out=mn[:], in_=pE,
                        axis=mybir.AxisListType.X, op=mybir.AluOpType.min,
                    )
                    s = psmall.tile([128, 1], F32, tag="s", name=f"s_{b}_{h}")
                    at = patt.tile([128, 256], F16, tag=f"att{h}", name=f"att_{b}_{h}")
                    nc.scalar.activation(
                        out=at[:], in_=pE,
                        func=mybir.ActivationFunctionType.Exp,
                        bias=mn[:], scale=-1.0, accum_out=s[:],
                    )
                    srec = psmall.tile([128, 1], F32, tag="srec", name=f"srec_{b}_{h}")
                    nc.vector.reciprocal(out=srec[:], in_=s[:])
                    nc.vector.tensor_mul(out=srec[:], in0=srec[:], in1=gg[b][:, h:h + 1])
                    ats = patt.tile([128, 256], F16, tag=f"atts{h}", name=f"atts_{b}_{h}")
                    nc.vector.tensor_scalar_mul(out=ats[:], in0=at[:], scalar1=srec[:])
                    att.append(ats)

                # transpose scaled attention (fp16 PE transposes)
                attT[b] = patt.tile([128, 2, 256], F16, tag="attT", name=f"attT_{b}")
                for j in range(2):
                    attT_ps = ppsX.tile([128, 256], F16, tag="psx", name=f"attTps_{b}_{j}")
                    for h in range(2):
                        nc.tensor.transpose(
                            attT_ps[:, 128 * h:128 * (h + 1)],
                            att[h][:, 128 * j:128 * (j + 1)],
                            ident16[:],
                        )
                    nc.vector.tensor_copy(out=attT[b][:, j, :], in_=attT_ps[:])

            def emit_phase2_pair(b, h, n0, k):
                # 1024 output columns of row-half h, as two 512 slabs. Most
                # slabs drain on the DVE with the residual fused into the
                # scalar_tensor_tensor (free there); every 4th slab adds x on
                # the PE (identity matmul) and drains with a plain ACT copy,
                # keeping the DVE off the critical path.
                o_sb = pout.tile([128, PAIR], F16, tag="osb", name=f"o_{b}_{h}_{n0}")
                for t in range(PAIR // NT):
                    c0 = t * NT
                    on_act = ((2 * k + t) % 3 == 2)
                    pso = ppsO.tile([128, NT], F32, tag="ps_o",
                                    name=f"pso_{b}_{h}_{n0}_{t}")
                    for j in range(2):
                        nc.tensor.matmul(
                            pso[:],
                            attT[b][:, j, 128 * h:128 * (h + 1)],
                            x16[b][j][:, n0 + c0:n0 + c0 + NT],
                            start=(j == 0), stop=(j == 1 and not on_act),
                        )
                    if on_act:
                        nc.tensor.matmul(
                            pso[:],
                            ident16[:],
                            x16[b][h][:, n0 + c0:n0 + c0 + NT],
                            start=False, stop=True,
                        )
                        nc.scalar.copy(out=o_sb[:, c0:c0 + NT], in_=pso[:])
                    else:
                        nc.vector.scalar_tensor_tensor(
                            out=o_sb[:, c0:c0 + NT], in0=pso[:], scalar=1.0,
                            in1=x16[b][h][:, n0 + c0:n0 + c0 + NT],
                            op0=mybir.AluOpType.mult, op1=mybir.AluOpType.add,
                        )
                nc.sync.dma_start(
                    out=out_d[b, 128 * h:128 * (h + 1), n0:n0 + PAIR],
                    in_=o_sb[:],
                )

            def phase2_pairs(b):
                k = 0
                for h in range(2):
                    for n0 in range(0, N, PAIR):
                        yield (b, h, n0, k)
                        k += 1

            GROUP_SEGS = {v: k for k, v in SEG_FIRST_GROUP.items()}  # gi -> seg

            # sample 0: remaining load segs (the first EARLY_SEGS were
            # emitted before the parameter prep); phase-1 runs with a
            # one-group lag (transposes of group i, then matmuls of group
            # i-1) so the PE never waits on the DVE's PSUM->SBUF copy
            for g in range(EARLY_SEGS, NSEG):
                emit_load_seg(0, g)
            alloc_psE(0)
            emit_phase1_T(0, 0)
            for gi in range(1, NGRP):
                emit_phase1_T(0, gi)
                emit_phase1_M(0, gi - 1)
                if gi == NGRP - 2:
                    emit_se(0)
            emit_phase1_M(0, NGRP - 1)
            # prologue of sample-1 phase 1 keeps the PE busy (and the HAM
            # warm) while sample-0's softmax runs on ACT/DVE
            emit_load_start(1)
            alloc_psE(1)
            emit_load_seg(1, 0)
            emit_phase1_T(1, 0)
            emit_load_seg(1, 1)
            emit_phase1_T(1, 1)
            emit_phase1_M(1, 0)
            emit_softmax(0)
            # steady state: weave sample-1 loads + phase-1 groups with
            # sample-0 phase-2 pairs so every engine's in-order queue always
            # has ready work at the front
            # front-load sample-1 phase 1 (paced by its load DMAs) and weave
            # in only enough sample-0 pairs to fill the PE's load-wait gaps;
            # softmax(1) then runs as early as possible so the sample-1 output
            # stream can overlap the remaining sample-0 output DMAs
            p2q = list(phase2_pairs(0))
            for gi in range(2, NGRP):
                if gi in GROUP_SEGS:
                    emit_load_seg(1, GROUP_SEGS[gi])
                if len(p2q) > 2:
                    emit_phase2_pair(*p2q.pop(0))
                emit_phase1_T(1, gi)
                emit_phase1_M(1, gi - 1)
                if gi == NGRP - 2:
                    emit_se(1)
            emit_phase1_M(1, NGRP - 1)
            # keep the PE fed with ready pairs while sample-1's softmax runs
            for _ in range(2):
                emit_phase2_pair(*p2q.pop(0))
            emit_softmax(1)
            p2q += list(phase2_pairs(1))
            for args in p2q:
                emit_phase2_pair(*args)

    nc.finalize()
    return nc


_CACHE = {}


def get_nc():
    if "nc" not in _CACHE:
        _CACHE["nc"] = build_nc()
    return _CACHE["nc"]


def kernel_with_result(x, gamma, w1, b1, w2, b2, trace=False, **_ignored):
    x = np.asarray(x, dtype=np.float32)
    nc = get_nc()
    params = {
        "gamma": np.asarray(gamma, np.float32).reshape(1),
        "w1": np.asarray(w1, np.float32) * np.float32(1.0 / N),
        "b1": np.asarray(b1, np.float32),
        "w2": np.asarray(w2, np.float32),
        # negated: the sigmoid runs as 1/(1+exp(-z - b2)) with bias=-b2
        "b2": -np.asarray(b2, np.float32),
        "ident": np.eye(128, dtype=np.float32),
    }
    xr = x.reshape(B, C, N)
    in_maps = [dict(params, x=xr[i * BL:(i + 1) * BL]) for i in range(NCORES)]
    res = run_bass_kernel_spmd(nc, in_maps, core_ids=list(range(NCORES)), trace=trace)
    out = np.concatenate(
        [np.asarray(res.results[i]["out"], dtype=np.float32) for i in range(NCORES)],
        axis=0,
    )
    return out.reshape(B, C, H, W), res


def kernel(x, gamma, w1, b1, w2, b2, **_ignored):
    out, _res = kernel_with_result(x, gamma, w1, b1, w2, b2, trace=False)
    return out
